# revision 14
# baseline (speedup 1.0000x reference)
"""MultiHeadAttention (relu pre-act, softmax, output proj + relu) on 8
Trainium2 NeuronCores via Bass/Tile.

Sharding: each core owns 512 query rows (S/4) of one batch (B=2 -> 4 cores
per batch) across ALL 16 heads; k/v of the batch are replicated on its 4
cores. The output projection is then fully local (no cross-device
reduction) -- the host only concatenates the 8 output slices.

Per-core dataflow (all DMAs 128-partition-wide, 4 heads per transfer):
  S^T[k,q] = relu(kT).T @ relu(qT)      PE fp8 DoubleRow, PSUM st[128,4,512]
  P^T      = exp(S^T/8)                 split ACT(exp) / DVE+Pool(Schraudolph)
  pv       = P^T.T @ [relu(V) | 1]      PE bf16, out [128 q, 65] per q-chunk
                                        (free dim 65 -> 2x cheaper than 512)
  am_q     = pv[:, :64] / pv[:, 64]     DVE recip + Pool broadcast-mult
  am^T     = transpose(am_q)            PE is_transpose matmuls (53ns each)
  outT     = relu(woT.T @ am + b)       PE bf16 + DVE bias, DMA out
"""

import sys

import numpy as np

try:
    import concourse.bass as bass
except ImportError:  # containers ship the repo here
    sys.path.insert(0, "/opt/trn_rl_repo")
    import concourse.bass as bass

import ml_dtypes

import concourse.mybir as mybir
import concourse.tile as tile
from concourse import bacc
from concourse.bass_utils import run_bass_kernel_spmd

B, S, D, H, DH = 2, 2048, 1024, 16, 64
# exp(s/8) = 2^(s*0.18034): bf16 Schraudolph constants for DVE/Pool path
SCHRAU_A = 0.125 * 1.4426950408889634 * 128.0
SCHRAU_B = 16256.0 - 5.5

NCORES = 8
SC = S // (NCORES // B)  # 512 query rows per core
NKC = S // 128  # 16 key chunks
HG = H // 4  # 4-head DMA groups
BF16 = mybir.dt.bfloat16
FP32 = mybir.dt.float32
FP8 = mybir.dt.float8e4
U16 = mybir.dt.uint16

# exp engine assignment per head parity: 8 chunk-pairs -> A(CT)/D(VE)/P(ool)
# quotas per 2 heads: ACT 7, Pool 6, DVE 3 (balanced against PE 7306ns)
import os as _os
_pat = _os.environ.get("EXP_PAT", "AADADADA,AADADADA,AADADADA,ADADADAD")
EXP_PATTERN = _pat.split(",")
TR_FP32 = _os.environ.get("TR_FP32", "0") == "1"

LAST_RESULTS = None  # BassKernelResults of the most recent run (for test.py)
_CACHED_NC = None


def _build_nc():
    nc = bacc.Bacc("TRN2", target_bir_lowering=False, debug=False)

    qT_d = nc.dram_tensor("qT", [HG, 128, 2, SC], BF16, kind="ExternalInput").ap()
    kT_d = nc.dram_tensor("kT", [HG, 128, 2, S], BF16, kind="ExternalInput").ap()
    v_d = nc.dram_tensor("v", [HG, 128, 4, NKC, DH], BF16, kind="ExternalInput").ap()
    woT_d = nc.dram_tensor("woT", [128, 8, D], BF16, kind="ExternalInput").ap()
    wob_d = nc.dram_tensor("wob", [128, 8], FP32, kind="ExternalInput").ap()
    ident_d = nc.dram_tensor("ident", [128, 128], FP32 if TR_FP32 else BF16, kind="ExternalInput").ap()
    outT_d = nc.dram_tensor("outT", [D, SC], FP32, kind="ExternalOutput").ap()
    DEBUG_AM = _os.environ.get("DEBUG_AM", "0") == "1"
    if DEBUG_AM:
        amdbg_d = nc.dram_tensor("amdbg", [128, 8, SC], BF16, kind="ExternalOutput").ap()

    AF = mybir.ActivationFunctionType
    ALU = mybir.AluOpType
    DR = mybir.MatmulPerfMode.DoubleRow

    with tile.TileContext(nc) as tc:
        with (
            tc.tile_pool(name="const", bufs=1) as cpool,
            tc.tile_pool(name="io", bufs=2) as iopool,
            tc.tile_pool(name="pt", bufs=12) as ptpool,
            tc.tile_pool(name="sm", bufs=2) as smpool,
            tc.tile_pool(name="persist", bufs=1) as perpool,
            tc.tile_pool(name="outp", bufs=2) as outpool,
            tc.tile_pool(name="psum", bufs=1, space="PSUM") as pspool,
        ):
            w_sb = cpool.tile([128, 8, D], BF16)  # w_sb[p,c,o] = woT[c*128+p, o]
            nc.sync.dma_start(out=w_sb, in_=woT_d)
            bias_sb = cpool.tile([128, 8], FP32)
            nc.sync.dma_start(out=bias_sb, in_=wob_d)
            ident = cpool.tile([128, 128], FP32 if TR_FP32 else BF16)
            nc.sync.dma_start(out=ident, in_=ident_d)

            # merged attn^T [d_in-part, chunk, query]; head h -> partitions
            # 64*(h%2) of chunk h//2. Persists until the projection.
            am_sb = perpool.tile([128, 8, SC], BF16)

            # [relu(V) | ones] double-buffered manually; ones column written
            # once per buffer, V relu'd into cols 0:64 each head.
            v_exts = [
                perpool.tile([128, NKC, DH + 1], BF16, name=f"v_ext{i}")
                for i in range(2)
            ]
            for ve in v_exts:
                nc.gpsimd.memset(ve[:, :, DH : DH + 1], 1.0)

            # score PSUM: 4 slots, chunk c -> slot c%4 (2 pairs in flight)
            st = pspool.tile([128, 4, SC], FP32)

            # raw input tiles per 4-head group (double-buffered via pool)
            def dma_group(g):
                kr = iopool.tile([128, 2, S], BF16, tag="kT_raw")
                nc.sync.dma_start(out=kr, in_=kT_d[g])
                qr = iopool.tile([128, 2, SC], BF16, tag="qT_raw")
                nc.sync.dma_start(out=qr, in_=qT_d[g])
                vr = iopool.tile([128, 4, NKC, DH], BF16, tag="v_raw")
                nc.sync.dma_start(out=vr, in_=v_d[g])
                return kr, qr, vr

            def relu_group(kr, qr, eng=None):
                eng = eng or nc.gpsimd
                k8 = iopool.tile([128, 2, S], FP8, tag="kT_f8")
                # group 0 split in halves so the first QK starts sooner
                if eng is nc.vector:
                    eng.tensor_scalar_max(
                        out=k8[:, :, 0 : S // 2], in0=kr[:, :, 0 : S // 2], scalar1=0.0
                    )
                    eng.tensor_scalar_max(
                        out=k8[:, :, S // 2 :], in0=kr[:, :, S // 2 :], scalar1=0.0
                    )
                else:
                    eng.tensor_scalar_max(out=k8, in0=kr, scalar1=0.0)
                q8 = iopool.tile([128, 2, SC], FP8, tag="qT_f8")
                eng.tensor_scalar_max(out=q8, in0=qr, scalar1=0.0)
                return k8, q8

            cur_raw = dma_group(0)
            cur_f8 = relu_group(cur_raw[0], cur_raw[1], eng=nc.vector)
            nxt_raw = nxt_f8 = None

            prev_tr = None  # (tr_ps, h) of previous head, transposed after QK

            for h in range(H):
                g, j = divmod(h, 4)
                if j == 0 and g + 1 < HG:
                    nxt_raw = dma_group(g + 1)
                if j == 2 and nxt_raw is not None:
                    nxt_f8 = relu_group(nxt_raw[0], nxt_raw[1])

                kr, qr, vr = cur_raw
                k8, q8 = cur_f8

                # relu(V) for this head into the alternating v_ext buffer
                ve = v_exts[h % 2]
                nc.gpsimd.tensor_scalar_max(
                    out=ve[:, :, 0:DH], in0=vr[:, j], scalar1=0.0
                )

                # QK^T: 16 chunks, fp8 DoubleRow (K=64 packed [32,2])
                p0 = 32 * j
                for c in range(NKC):
                    nc.tensor.matmul(
                        st[:, c % 4, :],
                        lhsT=k8[p0 : p0 + 32, :, c * 128 : (c + 1) * 128],
                        rhs=q8[p0 : p0 + 32, :, :],
                        start=True,
                        stop=True,
                        perf_mode=DR,
                        tile_position=(p0, 0),
                    )
                    # transposes of the previous head ride between QK and PV
                    if c == 3 and prev_tr is not None:
                        tr_p, am_qp, hp = prev_tr
                        for qc in range(4):
                            nc.tensor.matmul(
                                tr_p[:, qc, :],
                                lhsT=am_qp[:, qc, :],
                                rhs=ident,
                                is_transpose=True,
                            )
                        nc.vector.tensor_copy(
                            am_sb[64 * (hp % 2) : 64 * (hp % 2) + 64, hp // 2, :],
                            tr_p.rearrange("p a b -> p (a b)"),
                        )
                        prev_tr = None

                # exp of the 8 chunk-pairs, split across ACT/DVE/Pool
                pts = []
                pat = EXP_PATTERN[h % len(EXP_PATTERN)]
                for p in range(8):
                    sl = (2 * p) % 4
                    eng = pat[p]
                    if eng == "A":
                        pt = ptpool.tile([128, 2, SC], BF16, tag="pt")
                        nc.scalar.activation(
                            pt, st[:, sl : sl + 2, :], AF.Exp, scale=0.125
                        )
                    else:
                        ptu = ptpool.tile([128, 2, SC], U16, tag="pt")
                        e = nc.vector if eng == "D" else nc.gpsimd
                        e.tensor_scalar(
                            out=ptu,
                            in0=st[:, sl : sl + 2, :],
                            scalar1=SCHRAU_A,
                            scalar2=SCHRAU_B,
                            op0=ALU.mult,
                            op1=ALU.add,
                        )
                        pt = ptu.bitcast(BF16)
                    pts.append(pt)

                # PV: out [128 q, 65] per q-chunk, accumulated over 16 kc.
                # qc-major: accumulation groups must not interleave within a
                # PSUM tile (interleaving corrupts partials).
                acc = pspool.tile([128, 4, 128], FP32, tag="acc", bufs=2)
                for qc in range(4):
                    for kc in range(NKC):
                        pt = pts[kc // 2][:, kc % 2, :]
                        nc.tensor.matmul(
                            acc[:, qc, 0 : DH + 1],
                            lhsT=pt[:, qc * 128 : (qc + 1) * 128],
                            rhs=ve[:, kc, :],
                            start=(kc == 0),
                            stop=(kc == NKC - 1),
                        )

                # normalize: am_q[q, qc, dh] = pv / sumexp
                rz = smpool.tile([128, 4], FP32, tag="rz")
                nc.vector.reciprocal(rz, acc[:, :, DH])
                am_q = smpool.tile([128, 4, DH], FP32 if TR_FP32 else BF16, tag="am_q")
                nc.vector.tensor_tensor(
                    out=am_q,
                    in0=acc[:, :, 0:DH],
                    in1=rz.unsqueeze(2).broadcast_to([128, 4, DH]),
                    op=ALU.mult,
                )
                tr_p = pspool.tile([64, 4, 128], FP32 if TR_FP32 else BF16, tag="tr", bufs=2)
                prev_tr = (tr_p, am_q, h)

                if j == 3:
                    cur_raw, cur_f8 = nxt_raw, nxt_f8
                    nxt_raw = nxt_f8 = None

            # last head's transpose + merge
            tr_p, am_qp, hp = prev_tr
            for qc in range(4):
                nc.tensor.matmul(
                    tr_p[:, qc, :],
                    lhsT=am_qp[:, qc, :],
                    rhs=ident,
                    is_transpose=True,
                )
            nc.vector.tensor_copy(
                am_sb[64 * (hp % 2) : 64 * (hp % 2) + 64, hp // 2, :],
                tr_p.rearrange("p a b -> p (a b)"),
            )

            if DEBUG_AM:
                nc.sync.dma_start(out=amdbg_d, in_=am_sb)

            # output projection: outT = relu(woT.T @ am + b)
            for ot in range(8):
                pr = pspool.tile([128, 4, 128], FP32, tag="acc", bufs=2)
                prf = pr.rearrange("p a b -> p (a b)")
                for ic in range(8):
                    nc.tensor.matmul(
                        prf,
                        lhsT=w_sb[:, ic, ot * 128 : (ot + 1) * 128],
                        rhs=am_sb[:, ic, :],
                        start=(ic == 0),
                        stop=(ic == 7),
                    )
                o_sb = outpool.tile([128, SC], FP32, tag="osb")
                # relu(x + bias[o]) in one DVE pass; bias is per-partition.
                nc.vector.tensor_scalar(
                    out=o_sb,
                    in0=prf,
                    scalar1=bias_sb[:, ot : ot + 1],
                    scalar2=0.0,
                    op0=ALU.add,
                    op1=ALU.max,
                )
                nc.sync.dma_start(
                    out=outT_d[ot * 128 : (ot + 1) * 128, :], in_=o_sb
                )

    nc.compile()
    return nc


def kernel(q, k, v, w_o_w, w_o_b):
    global LAST_RESULTS, _CACHED_NC

    q = np.asarray(q, dtype=np.float32)
    k = np.asarray(k, dtype=np.float32)
    v = np.asarray(v, dtype=np.float32)
    w_o_w = np.asarray(w_o_w, dtype=np.float32)
    w_o_b = np.asarray(w_o_b, dtype=np.float32)

    bf = ml_dtypes.bfloat16
    # [B,S,D] -> [B,H,DH,S] per-head transposed, fp8-DoubleRow paired
    # [B, HG, 128, 2, S]: head 4g+j at partitions 32j:32j+32
    qT = np.ascontiguousarray(
        q.reshape(B, S, H, DH).transpose(0, 2, 3, 1).astype(bf)
    ).reshape(B, HG, 128, 2, S)
    kT = np.ascontiguousarray(
        k.reshape(B, S, H, DH).transpose(0, 2, 3, 1).astype(bf)
    ).reshape(B, HG, 128, 2, S)
    # v: [B, HG, 128(key-in-chunk), 4(j), NKC, DH]
    vh = np.ascontiguousarray(
        v.reshape(B, NKC, 128, HG, 4, DH).transpose(0, 3, 2, 4, 1, 5).astype(bf)
    )
    woT = np.ascontiguousarray(
        w_o_w.T.reshape(8, 128, D).transpose(1, 0, 2).astype(bf)
    )
    wob = np.ascontiguousarray(w_o_b.reshape(8, 128).T)  # [128, 8] fp32
    import os as _os2
    ident = np.eye(128, dtype=np.float32 if _os2.environ.get("TR_FP32","0")=="1" else bf)

    if _CACHED_NC is None:
        _CACHED_NC = _build_nc()
    nc = _CACHED_NC

    in_maps = []
    for c in range(NCORES):
        b = c // (NCORES // B)
        s0 = (c % (NCORES // B)) * SC
        in_maps.append(
            {
                "qT": np.ascontiguousarray(qT[b, ..., s0 : s0 + SC]),
                "kT": kT[b],
                "v": vh[b],
                "woT": woT,
                "wob": wob,
                "ident": ident,
            }
        )

    LAST_RESULTS = run_bass_kernel_spmd(nc, in_maps, core_ids=list(range(NCORES)))

    out = np.empty((B, S, D), dtype=np.float32)
    for c in range(NCORES):
        b = c // (NCORES // B)
        s0 = (c % (NCORES // B)) * SC
        out[b, s0 : s0 + SC, :] = LAST_RESULTS.results[c]["outT"].T
    return out


# revision 20
# speedup vs baseline: 1.8422x; 1.8422x over previous
"""MultiHeadAttention (relu pre-act, softmax, output proj + relu) on 8
Trainium2 NeuronCores via Bass/Tile.

Sharding: each core owns 512 query rows (S/4) of one batch (B=2 -> 4 cores
per batch) across ALL 16 heads; k/v of the batch are replicated on its 4
cores. The output projection is then fully local (no cross-device
reduction) -- the host only concatenates the 8 output slices.

Per-core dataflow (all input DMAs 128-partition-wide, 4 heads per
transfer; head j of a group lives on partitions 32j:32j+32):
  S^T[k,q] = relu(kT).T @ relu(qT)   PE fp8 DoubleRow, PSUM pairs (bufs=3)
  P^T      = exp(S^T/8)              split: ACT exp / DVE Schraudolph 2^y
  pvT      = [relu(V)|1s]^T @ P^T    PE bf16, out[0:64]=attnT,
                                     out[64:128]=sumexp replicated 64x
  am       = pvT * recip(sumexp)     DVE, written straight into am_sb
  outT     = relu(woT.T @ am + b)    PE bf16 + DVE bias, DMA out
"""

import os as _os
import sys

import numpy as np

try:
    import concourse.bass as bass
except ImportError:  # containers ship the repo here
    sys.path.insert(0, "/opt/trn_rl_repo")
    import concourse.bass as bass

import ml_dtypes

import concourse.mybir as mybir
import concourse.tile as tile
from concourse import bacc
from concourse.bass_utils import run_bass_kernel_spmd

B, S, D, H, DH = 2, 2048, 1024, 16, 64
# exp(s/8) = 2^(s*0.18034): bf16 Schraudolph constants for the DVE path
SCHRAU_A = 0.125 * 1.4426950408889634 * 128.0
SCHRAU_B = 16256.0 - 5.5

NCORES = 8
SC = S // (NCORES // B)  # 512 query rows per core
NKC = S // 128  # 16 key chunks
HG = H // 4  # 4-head DMA groups
BF16 = mybir.dt.bfloat16
FP32 = mybir.dt.float32
FP8 = mybir.dt.float8e4
U16 = mybir.dt.uint16

# exp engine per chunk-pair: A(CT) or D(VE); ~5A/3D balances ACT against
# DVE's pair cost + recip/divide load, with PE (5120ns/head) the target.
_pat = _os.environ.get("EXP_PAT", "AADADADA,AADADADA,AADADADA,ADADADAD")
EXP_PATTERN = _pat.split(",")

LAST_RESULTS = None  # BassKernelResults of the most recent run (for test.py)
_CACHED_NC = None


def _build_nc():
    nc = bacc.Bacc("TRN2", target_bir_lowering=False, debug=False)

    qT_d = nc.dram_tensor("qT", [HG, 128, 2, SC], BF16, kind="ExternalInput").ap()
    kT_d = nc.dram_tensor("kT", [HG, 128, 2, S], BF16, kind="ExternalInput").ap()
    v_d = nc.dram_tensor("v", [HG, 128, 4, NKC, DH], BF16, kind="ExternalInput").ap()
    woT_d = nc.dram_tensor("woT", [128, 8, D], BF16, kind="ExternalInput").ap()
    wob_d = nc.dram_tensor("wob", [128, 8], FP32, kind="ExternalInput").ap()
    outT_d = nc.dram_tensor("outT", [D, SC], FP32, kind="ExternalOutput").ap()
    DEBUG_AM = _os.environ.get("DEBUG_AM", "0") == "1"
    if DEBUG_AM:
        amdbg_d = nc.dram_tensor("amdbg", [128, 8, SC], BF16, kind="ExternalOutput").ap()

    AF = mybir.ActivationFunctionType
    ALU = mybir.AluOpType
    DR = mybir.MatmulPerfMode.DoubleRow

    with tile.TileContext(nc) as tc:
        with (
            tc.tile_pool(name="const", bufs=1) as cpool,
            tc.tile_pool(name="io", bufs=2) as iopool,
            tc.tile_pool(name="pt", bufs=4) as ptpool,
            tc.tile_pool(name="sm", bufs=2) as smpool,
            tc.tile_pool(name="persist", bufs=1) as perpool,
            tc.tile_pool(name="outp", bufs=2) as outpool,
            tc.tile_pool(name="psum", bufs=1, space="PSUM") as pspool,
        ):
            # raw input tiles per 4-head group (double-buffered via pool);
            # group 0's kT arrives in halves so relu+QK can start early
            def dma_group(g, split=False):
                kr = iopool.tile([128, 2, S], BF16, tag="kT_raw")
                qr = iopool.tile([128, 2, SC], BF16, tag="qT_raw")
                nc.sync.dma_start(out=qr, in_=qT_d[g])
                if split:
                    nc.sync.dma_start(
                        out=kr[:, :, 0 : S // 2], in_=kT_d[g, :, :, 0 : S // 2]
                    )
                    nc.sync.dma_start(
                        out=kr[:, :, S // 2 :], in_=kT_d[g, :, :, S // 2 :]
                    )
                else:
                    nc.sync.dma_start(out=kr, in_=kT_d[g])
                vr = iopool.tile([128, 4, NKC, DH], BF16, tag="v_raw")
                nc.sync.dma_start(out=vr, in_=v_d[g])
                return kr, qr, vr

            # group 0+1 input DMAs go first so compute can ramp before the
            # big weight transfer hogs descriptor generation
            raws = {0: dma_group(0, split=True)}
            if HG > 1:
                raws[1] = dma_group(1)

            w_sb = cpool.tile([128, 8, D], BF16)  # w_sb[p,c,o] = woT[c*128+p, o]
            nc.sync.dma_start(out=w_sb, in_=woT_d)
            bias_sb = cpool.tile([128, 8], FP32)
            nc.sync.dma_start(out=bias_sb, in_=wob_d)

            # merged attn^T [d_in-part, chunk, query]; head h -> partitions
            # 64*(h%2) of chunk h//2. Persists until the projection.
            am_sb = perpool.tile([128, 8, SC], BF16)

            def relu_group(kr, qr, eng):
                k8 = iopool.tile([128, 2, S], FP8, tag="kT_f8")
                if eng is nc.vector:  # group 0: halves, so QK starts sooner
                    eng.tensor_scalar_max(
                        out=k8[:, :, 0 : S // 2], in0=kr[:, :, 0 : S // 2], scalar1=0.0
                    )
                    eng.tensor_scalar_max(
                        out=k8[:, :, S // 2 :], in0=kr[:, :, S // 2 :], scalar1=0.0
                    )
                else:
                    eng.tensor_scalar_max(out=k8, in0=kr, scalar1=0.0)
                q8 = iopool.tile([128, 2, SC], FP8, tag="qT_f8")
                eng.tensor_scalar_max(out=q8, in0=qr, scalar1=0.0)
                return k8, q8

            f8s = {0: relu_group(raws[0][0], raws[0][1], eng=nc.vector)}

            for h in range(H):
                g, j = divmod(h, 4)
                if j == 0 and g + 2 < HG:
                    raws[g + 2] = dma_group(g + 2)
                if j == 1 and g + 1 < HG and g + 1 not in f8s:
                    f8s[g + 1] = relu_group(
                        raws[g + 1][0], raws[g + 1][1], eng=nc.gpsimd
                    )

                vr = raws[g][2]
                k8, q8 = f8s[g]

                # [relu(V) | ones-64] for this head: lhsT of the PV matmul;
                # the 64 ones columns replicate sumexp across partitions
                # 64:128 of the PV output so the divide is per-partition.
                ve = iopool.tile([128, NKC, 2 * DH], BF16, tag="v_ext")
                nc.vector.tensor_scalar_max(
                    out=ve[:, :, 0:DH], in0=vr[:, j], scalar1=0.0
                )
                nc.gpsimd.memset(ve[:, :, DH : 2 * DH], 1.0)

                # QK + exp + PV per chunk-pair; st pairs pool-rotate (bufs=3)
                p0 = 32 * j
                pat = EXP_PATTERN[h % len(EXP_PATTERN)]
                acc = pspool.tile([128, SC], FP32, tag="acc", bufs=2)
                prev_pv = None
                for p in range(8):
                    stp = pspool.tile([128, 2, SC], FP32, tag="st", bufs=3)
                    for i in (0, 1):
                        c = 2 * p + i
                        nc.tensor.matmul(
                            stp[:, i, :],
                            lhsT=k8[p0 : p0 + 32, :, c * 128 : (c + 1) * 128],
                            rhs=q8[p0 : p0 + 32, :, :],
                            start=True,
                            stop=True,
                            perf_mode=DR,
                            tile_position=(p0, 0),
                        )
                    if prev_pv is not None:
                        pv_pt, pv_p = prev_pv
                        for i in (0, 1):
                            kc = 2 * pv_p + i
                            nc.tensor.matmul(
                                acc,
                                lhsT=ve[:, kc, :],
                                rhs=pv_pt[:, i, :],
                                start=(kc == 0),
                                stop=(kc == NKC - 1),
                            )
                    if pat[p] == "A":
                        pt = ptpool.tile([128, 2, SC], BF16, tag="pt")
                        nc.scalar.activation(pt, stp, AF.Exp, scale=0.125)
                    else:
                        ptu = ptpool.tile([128, 2, SC], U16, tag="pt")
                        nc.vector.tensor_scalar(
                            out=ptu,
                            in0=stp,
                            scalar1=SCHRAU_A,
                            scalar2=SCHRAU_B,
                            op0=ALU.mult,
                            op1=ALU.add,
                        )
                        pt = ptu.bitcast(BF16)
                    prev_pv = (pt, p)
                pv_pt, pv_p = prev_pv
                for i in (0, 1):
                    kc = 2 * pv_p + i
                    nc.tensor.matmul(
                        acc,
                        lhsT=ve[:, kc, :],
                        rhs=pv_pt[:, i, :],
                        start=(kc == 0),
                        stop=(kc == NKC - 1),
                    )

                # am = attnT / sumexp, straight into the merged buffer
                rd = smpool.tile([DH, SC], FP32, tag="rd")
                nc.vector.reciprocal(rd, acc[DH : 2 * DH, :])
                r0 = 64 * (h % 2)
                nc.vector.tensor_tensor(
                    out=am_sb[r0 : r0 + DH, h // 2, :],
                    in0=acc[0:DH, :],
                    in1=rd,
                    op=ALU.mult,
                )

                if j == 3:
                    del raws[g], f8s[g]

            if DEBUG_AM:
                nc.sync.dma_start(out=amdbg_d, in_=am_sb)

            # output projection: outT = relu(woT.T @ am + b)
            for ot in range(8):
                pr = pspool.tile([128, SC], FP32, tag="acc", bufs=2)
                for ic in range(8):
                    nc.tensor.matmul(
                        pr,
                        lhsT=w_sb[:, ic, ot * 128 : (ot + 1) * 128],
                        rhs=am_sb[:, ic, :],
                        start=(ic == 0),
                        stop=(ic == 7),
                    )
                o_sb = outpool.tile([128, SC], FP32, tag="osb")
                # relu(x + bias[o]) in one DVE pass; bias is per-partition.
                nc.vector.tensor_scalar(
                    out=o_sb,
                    in0=pr,
                    scalar1=bias_sb[:, ot : ot + 1],
                    scalar2=0.0,
                    op0=ALU.add,
                    op1=ALU.max,
                )
                nc.sync.dma_start(
                    out=outT_d[ot * 128 : (ot + 1) * 128, :], in_=o_sb
                )

    nc.compile()
    return nc


def kernel(q, k, v, w_o_w, w_o_b):
    global LAST_RESULTS, _CACHED_NC

    q = np.asarray(q, dtype=np.float32)
    k = np.asarray(k, dtype=np.float32)
    v = np.asarray(v, dtype=np.float32)
    w_o_w = np.asarray(w_o_w, dtype=np.float32)
    w_o_b = np.asarray(w_o_b, dtype=np.float32)

    bf = ml_dtypes.bfloat16
    # [B,S,D] -> [B,H,DH,S] per-head transposed, fp8-DoubleRow paired:
    # [B, HG, 128, 2, S] with head 4g+j on partitions 32j:32j+32
    qT = np.ascontiguousarray(
        q.reshape(B, S, H, DH).transpose(0, 2, 3, 1).astype(bf)
    ).reshape(B, HG, 128, 2, S)
    kT = np.ascontiguousarray(
        k.reshape(B, S, H, DH).transpose(0, 2, 3, 1).astype(bf)
    ).reshape(B, HG, 128, 2, S)
    # v: [B, HG, 128(key-in-chunk), 4(j), NKC, DH]
    vh = np.ascontiguousarray(
        v.reshape(B, NKC, 128, HG, 4, DH).transpose(0, 3, 2, 4, 1, 5).astype(bf)
    )
    woT = np.ascontiguousarray(
        w_o_w.T.reshape(8, 128, D).transpose(1, 0, 2).astype(bf)
    )
    wob = np.ascontiguousarray(w_o_b.reshape(8, 128).T)  # [128, 8] fp32

    if _CACHED_NC is None:
        _CACHED_NC = _build_nc()
    nc = _CACHED_NC

    in_maps = []
    for c in range(NCORES):
        b = c // (NCORES // B)
        s0 = (c % (NCORES // B)) * SC
        in_maps.append(
            {
                "qT": np.ascontiguousarray(qT[b, ..., s0 : s0 + SC]),
                "kT": kT[b],
                "v": vh[b],
                "woT": woT,
                "wob": wob,
            }
        )

    LAST_RESULTS = run_bass_kernel_spmd(nc, in_maps, core_ids=list(range(NCORES)))

    out = np.empty((B, S, D), dtype=np.float32)
    for c in range(NCORES):
        b = c // (NCORES // B)
        s0 = (c % (NCORES // B)) * SC
        out[b, s0 : s0 + SC, :] = LAST_RESULTS.results[c]["outT"].T
    return out
